# revision 23
# baseline (speedup 1.0000x reference)
"""Trainium2 Bass kernel for MultiHeadBinaryPositionIO.

Math rewrite: the reference computes
    values   = einsum('bsd,hed->bhse', x, Wv)        (huge)
    head_out = einsum('bhs,bhse->bhe', weights, values)
which reassociates to
    ctx      = einsum('bhs,bsd->bhd', weights, x)    (the only big op)
    head_out = einsum('bhd,hed->bhe', ctx, Wv)       (tiny)
The bit-match log-score is affine in the number of mismatched bits m
(in fp32, log(1 + 1e-8) == 0), so scores = m * k_h with
k_h = log(1e-8) * slope_h / T, and masked positions get m += 1e6 so
exp underflows to exactly 0 after max-subtraction.

Device work per core (2 of 16 batches): softmax over S=4096 for 32
(b,h) rows, PE transpose of the weights, and ctx^T = x^T @ w^T via 512
fp32 matmuls with x as the stationary operand. Everything else is
O(B*H*S) or smaller index/projection glue done on host.
"""

import sys

for _p in ("/opt/trn_rl_repo", "/root/.axon_site/_ro/trn_rl_repo"):
    if _p not in sys.path:
        sys.path.append(_p)

import numpy as np

import concourse.bass as bass
import concourse.bacc as bacc
import concourse.mybir as mybir
import concourse.tile as tile
from concourse import bass_utils

B, S, D, H = 16, 4096, 1024, 16
NCORES = 8
BL = B // NCORES            # batches per core = 2
ROWS = BL * H               # softmax rows per core = 32
KCH = S // 128              # 32 contraction chunks of 128 positions
NKK = KCH // 2              # 16 x-tiles of [128, 2048] (1 MiB)
NDT = D // 128              # 8 output d-tiles
NUM_BITS = 12
TEMPERATURE = 0.1

F32 = mybir.dt.float32

_CACHE = {}


def build_nc(
    nkk_used=NKK,
    repeats=1,
    xbufs=4,
    xgrp=2,
    xeng="sync",
    mode="full",
    store_eng="gpsimd",
    small_eng="scalar",
    swq=1,
):
    nc = bacc.Bacc("TRN2", debug=False, num_devices=NCORES, num_swdge_queues=swq)

    x_d = nc.dram_tensor("x", [BL, S, D], F32, kind="ExternalInput")
    mp_d = nc.dram_tensor("mprime", [ROWS, S], F32, kind="ExternalInput")
    kv_d = nc.dram_tensor("kvec", [ROWS, 1], F32, kind="ExternalInput")
    id_d = nc.dram_tensor("ident", [32, 32], F32, kind="ExternalInput")
    w_d = nc.dram_tensor("w_out", [ROWS, S], F32, kind="ExternalOutput")
    c_d = nc.dram_tensor("ctxT", [128, BL * 128], F32, kind="ExternalOutput")

    with tile.TileContext(nc) as tc:
        with (
            tc.tile_pool(name="sm", bufs=1 if repeats == 1 else 2, space="SBUF") as sm,
            tc.tile_pool(name="xp", bufs=xbufs) as xp,
            tc.tile_pool(name="ps", bufs=8, space="PSUM") as psp,
        ):
          for rep in range(repeats):
            mp = sm.tile([ROWS, S], F32)
            kv = sm.tile([ROWS, 1], F32)
            ident = sm.tile([32, 32], F32)
            ld = getattr(nc, small_eng)
            st = getattr(nc, store_eng)
            ld.dma_start(mp[:], mp_d[:])
            ld.dma_start(kv[:], kv_d[:])
            ld.dma_start(ident[:], id_d[:])

            # scores = mprime * k_row;  softmax over the free (S) axis.
            # w is reused in-place: scores -> exp -> normalized weights.
            w = sm.tile([ROWS, S], F32)
            nc.vector.tensor_scalar_mul(w[:], mp[:], kv[:, 0:1])
            nmx = sm.tile([ROWS, 1], F32)
            nc.vector.reduce_max(
                nmx[:], w[:], axis=mybir.AxisListType.X, negate=True
            )
            sums = sm.tile([ROWS, 1], F32)
            nc.scalar.activation(
                w[:],
                w[:],
                mybir.ActivationFunctionType.Exp,
                bias=nmx[:, 0:1],
                scale=1.0,
                accum_out=sums[:, 0:1],
            )
            rs = sm.tile([ROWS, 1], F32)
            nc.vector.reciprocal(rs[:], sums[:])
            nc.vector.tensor_scalar_mul(w[:], w[:], rs[:, 0:1])
            st.dma_start(w_d[:], w[:])

            # Transpose w [32, S] into wT [128, 32*KCH]: chunk c holds
            # wT[p, 32c + r] = w[r, 128c + p].
            wT = sm.tile([128, 32 * KCH], F32)
            for c in range(KCH):
                pt = psp.tile([128, 32], F32, tag="ps")
                nc.tensor.transpose(pt[:], w[:, 128 * c : 128 * (c + 1)], ident[:])
                nc.vector.tensor_copy(wT[:, 32 * c : 32 * (c + 1)], pt[:])

            # ctxT[dd, b*128 + dt*16 + h] = sum_s x[b, s, 128dt+dd] * w[b*16+h, s]
            ctxT = sm.tile([128, BL * 128], F32)
            if mode == "front":
                nc.vector.memset(ctxT[:], 0.0)
                st.dma_start(c_d[:], ctxT[:])
                continue
            if mode == "dmaonly":
                junk = sm.tile([128, 4 * NKK * BL], F32)
                dma_eng = getattr(nc, xeng)
                for b in range(BL):
                    x_r = x_d[b].rearrange("(n p) d -> p n d", p=128)
                    for kk in range(nkk_used * 2 // xgrp):
                        xt = xp.tile([128, xgrp * D], F32)
                        dma_eng.dma_start(
                            xt[:].rearrange("p (n d) -> p n d", n=xgrp),
                            x_r[:, xgrp * kk : xgrp * (kk + 1), :],
                        )
                        nc.vector.tensor_copy(
                            junk[:, (b * NKK + kk) * 2 : (b * NKK + kk) * 2 + 2],
                            xt[:, 0:2],
                        )
                nc.vector.memset(ctxT[:], 0.0)
                st.dma_start(c_d[:], ctxT[:])
                continue
            if mode == "peonly":
                xt0 = sm.tile([128, xgrp * D], F32)
                x_r0 = x_d[0].rearrange("(n p) d -> p n d", p=128)
                nc.sync.dma_start(
                    xt0[:].rearrange("p (n d) -> p n d", n=xgrp),
                    x_r0[:, 0:xgrp, :],
                )
            for b in range(BL):
                x_r = x_d[b].rearrange("(n p) d -> p n d", p=128)  # [128, KCH, D]
                pms = [
                    psp.tile([128, 16], F32, tag="ps", name=f"pm_{b}_{dt}")
                    for dt in range(NDT)
                ]
                if xeng == "both":
                    dma_engs = [nc.sync, nc.scalar]
                else:
                    dma_engs = [getattr(nc, xeng)]
                for kk in range(nkk_used * 2 // xgrp):
                    if mode == "peonly":
                        xt = xt0
                    else:
                        dma_eng = dma_engs[kk % len(dma_engs)]
                        xt = xp.tile([128, xgrp * D], F32)
                        if xgrp == 1:
                            dma_eng.dma_start(xt[:], x_r[:, kk, :])
                        else:
                            dma_eng.dma_start(
                                xt[:].rearrange("p (n d) -> p n d", n=xgrp),
                                x_r[:, xgrp * kk : xgrp * (kk + 1), :],
                            )
                    for sub in range(xgrp):
                        k = xgrp * kk + sub
                        for dt in range(NDT):
                            nc.tensor.matmul(
                                pms[dt][:, :],
                                lhsT=xt[:, D * sub + 128 * dt : D * sub + 128 * (dt + 1)],
                                rhs=wT[:, 32 * k + 16 * b : 32 * k + 16 * b + 16],
                                start=(k == 0),
                                stop=(k == 2 * nkk_used - 1),
                            )
                for dt in range(NDT):
                    nc.vector.tensor_copy(
                        ctxT[:, 128 * b + 16 * dt : 128 * b + 16 * (dt + 1)],
                        pms[dt][:],
                    )
            st.dma_start(c_d[:], ctxT[:])

    nc.compile()
    return nc


def get_nc():
    if "nc" not in _CACHE:
        _CACHE["nc"] = build_nc()
    return _CACHE["nc"]


def host_prep(positions, anchor, read_offset):
    """mprime [B,S] f32 (mismatch count, +1e6 where masked) and kvec [ROWS] f32."""
    pos = np.asarray(positions).astype(np.int64)
    anc = np.asarray(anchor).astype(np.int64)
    ro = np.asarray(read_offset).astype(np.int64)

    rel = np.clip(pos - anc[:, None] - 1, 0, 2**NUM_BITS - 1)
    xor = rel ^ ro[:, None]
    m = np.zeros(pos.shape, np.int64)
    for k in range(NUM_BITS):
        m += (xor >> k) & 1
    mask = pos > anc[:, None]
    mprime = m.astype(np.float32) + np.where(mask, 0.0, 1e6).astype(np.float32)

    slopes = 2.0 ** (-8.0 / H * (np.arange(H, dtype=np.float32) + 1.0))
    L = np.log(np.float32(1e-8)).astype(np.float64)
    k_h = (L * slopes.astype(np.float64) / TEMPERATURE).astype(np.float32)
    kvec = np.concatenate([k_h] * BL)  # rows are (b_local, h), b-major
    return mprime, kvec


def make_in_maps(x, mprime, kvec):
    ident = np.eye(32, dtype=np.float32)
    kv = np.ascontiguousarray(kvec.reshape(ROWS, 1))
    in_maps = []
    for c in range(NCORES):
        bsl = slice(BL * c, BL * (c + 1))
        in_maps.append(
            {
                "x": np.ascontiguousarray(x[bsl]),
                "mprime": np.ascontiguousarray(np.repeat(mprime[bsl], H, axis=0)),
                "kvec": kv,
                "ident": ident,
            }
        )
    return in_maps


def assemble(results, Wv, Wo, Wc, read_offset):
    weights = np.empty((B, H, 1, S), np.float32)
    ctx = np.empty((B, H, D), np.float32)
    for c in range(NCORES):
        r = results[c]
        bsl = slice(BL * c, BL * (c + 1))
        weights[bsl, :, 0, :] = r["w_out"].reshape(BL, H, S)
        ct = r["ctxT"].reshape(128, BL, NDT, 16)  # [dd, b, dt, h]
        ctx[bsl] = ct.transpose(1, 3, 2, 0).reshape(BL, H, D)

    head_out = np.einsum("bhd,hed->bhe", ctx, np.asarray(Wv, np.float32))
    multi_head = head_out.reshape(B, D)
    out = multi_head @ np.asarray(Wo, np.float32).T
    char_value = out @ np.asarray(Wc, np.float32).T

    ro = np.asarray(read_offset)
    new_offset = ro + np.asarray(1, dtype=ro.dtype)
    return char_value, new_offset, weights


def _get_sharded():
    """Cached jitted shard_map executable over the 8 cores (axon PJRT path)."""
    if "sharded" in _CACHE:
        return _CACHE["sharded"]
    import jax
    from jax.experimental.shard_map import shard_map
    from jax.sharding import Mesh, NamedSharding, PartitionSpec

    from concourse import bass2jax

    nc = get_nc()
    bass2jax.install_neuronx_cc_hook()
    partition_name = nc.partition_id_tensor.name if nc.partition_id_tensor else None
    in_names, out_names, out_avals = [], [], []
    for alloc in nc.m.functions[0].allocations:
        if not isinstance(alloc, mybir.MemoryLocationSet):
            continue
        name = alloc.memorylocations[0].name
        if alloc.kind == "ExternalInput":
            if name != partition_name:
                in_names.append(name)
        elif alloc.kind == "ExternalOutput":
            out_names.append(name)
            out_avals.append(
                jax.core.ShapedArray(
                    tuple(alloc.tensor_shape), mybir.dt.np(alloc.dtype)
                )
            )
    all_in_names = in_names + out_names + ([partition_name] if partition_name else [])

    def _body(*args):
        operands = list(args)
        if partition_name is not None:
            operands.append(bass2jax.partition_id_tensor())
        return tuple(
            bass2jax._bass_exec_p.bind(
                *operands,
                out_avals=tuple(out_avals),
                in_names=tuple(all_in_names),
                out_names=tuple(out_names),
                lowering_input_output_aliases=(),
                sim_require_finite=True,
                sim_require_nnan=True,
                nc=nc,
            )
        )

    devices = jax.devices()[:NCORES]
    mesh = Mesh(np.asarray(devices), ("core",))
    nio = len(in_names) + len(out_names)
    sharded = jax.jit(
        shard_map(
            _body,
            mesh=mesh,
            in_specs=(PartitionSpec("core"),) * nio,
            out_specs=(PartitionSpec("core"),) * len(out_names),
            check_rep=False,
        ),
        keep_unused=True,
    )
    sh = NamedSharding(mesh, PartitionSpec("core"))
    _CACHE["sharded"] = (sharded, sh, in_names, out_names, out_avals, jax)
    return _CACHE["sharded"]


def _run_fast(x, mprime, kvec):
    """Zero-copy dispatch: x is already core-major along batch."""
    sharded, sh, in_names, out_names, out_avals, jax = _get_sharded()
    full = {
        "x": x,
        "mprime": np.repeat(mprime, H, axis=0),
        "kvec": np.tile(kvec.reshape(ROWS, 1), (NCORES, 1)),
        "ident": np.tile(np.eye(32, dtype=np.float32), (NCORES, 1)),
    }
    args = [jax.device_put(full[nm], sh) for nm in in_names]
    zeros = [
        jax.device_put(np.zeros((NCORES * a.shape[0], *a.shape[1:]), a.dtype), sh)
        for a in out_avals
    ]
    outs = sharded(*args, *zeros)
    results = []
    for c in range(NCORES):
        results.append(
            {
                nm: np.asarray(outs[i]).reshape(NCORES, *out_avals[i].shape)[c]
                for i, nm in enumerate(out_names)
            }
        )
    return results


def kernel(**inputs):
    x = np.ascontiguousarray(np.asarray(inputs["x"], np.float32))
    mprime, kvec = host_prep(
        inputs["positions"], inputs["anchor"], inputs["read_offset"]
    )
    try:
        results = _run_fast(x, mprime, kvec)
    except Exception:
        in_maps = make_in_maps(x, mprime, kvec)
        res = bass_utils.run_bass_kernel_spmd(
            get_nc(), in_maps, core_ids=list(range(NCORES))
        )
        results = res.results
    return assemble(
        results, inputs["Wv"], inputs["Wo"], inputs["Wc"], inputs["read_offset"]
    )
